# revision 5
# baseline (speedup 1.0000x reference)
"""MLA (multi-head latent attention) Trainium2 kernel, 8-core SPMD.

Strategy (hardcoded for B=2, S=2048, DIM=2048, NH=16, HD=128, HDR=64,
DCKV=512, DCQ=1536):
  - Token-shard (flattened b*s, 512 tok/core) the low-rank down-projections
    (dq/dkv/kr + rope on kr), all feature-major (transposed) so matmuls need
    no on-device transposes.
  - AllGather the bundle [c_qT | c_kvT | k_rT] (bf16).
  - Head-shard: core c owns attention heads {c, c+8}. The reference views
    concat([uq, r_q]) as 16 heads x 192 dims, so head h's q/k window is
    cols [192h, 192h+192) of the concat. We decompose each head uniformly:
    main 128 dims + ext 64 dims + rope 64 dims, with host-built (possibly
    zero) weight slices; rope-only heads get the 3 relevant W_qr 64-row
    blocks PRE-SUMMED (valid because r_k is broadcast across rope blocks
    and rope is linear + identical per block).
  - Transpose-free attention: scoresT [k-tokens x q-tokens], exp without
    max-subtraction (scores are O(1) for this problem's scale), causal mask
    by 0/1 bf16 multiply, row-sums via ones-matmul, normalize after AV.
  - AllToAll y back to token sharding, local out-proj with full W_out.
  - bf16 matmul inputs (fp32 matmul is 4x slower on TRN2), fp32 PSUM.
"""
import sys

sys.path.insert(0, "/opt/trn_rl_repo")

import numpy as np
import ml_dtypes

import concourse.bass as bass
import concourse.mybir as mybir
import concourse.tile as tile
from concourse import bacc
from concourse.bass_utils import run_bass_kernel_spmd

BF = ml_dtypes.bfloat16
F32 = mybir.dt.float32
BF16 = mybir.dt.bfloat16

B, S, DIM = 2, 2048, 2048
NH, HD, HDR = 16, 128, 64
DCKV, DCQ = 512, 1536
R = 8            # cores
TL = 512         # tokens per core (flattened B*S / R)
T = B * S        # 4096
HPC = 2          # heads per core: {c, c+8}
NKQ = DCQ // 128   # 12 contraction chunks for c_q
NKD = DIM // 128   # 16 for x
NKC = DCKV // 128  # 4 for c_kv
BUND = DCQ + DCKV + HDR  # 2112 bundle rows

DEBUG = False


def _rope_rows(nc, out_ap, src_ap, cos_lo, cos_hi, sin_lo, sin_hi, tmp_pool):
    """rope on 64 feature-major rows: src/out [64, W] (out may be 2 slices).
    out[0:32]  = src[0:32]*cos_lo - src[32:64]*sin_lo
    out[32:64] = src[32:64]*cos_hi + src[0:32]*sin_hi
    src_ap: callable idx->AP for row slices (so PSUM sources work);
    out_ap: callable likewise."""
    W = cos_lo.shape[-1]
    t0 = tmp_pool.tile([32, W], F32, tag="rope_t0")
    t1 = tmp_pool.tile([32, W], F32, tag="rope_t1")
    nc.vector.tensor_mul(t0[:], src_ap(0), cos_lo)
    nc.vector.tensor_mul(t1[:], src_ap(1), sin_lo)
    nc.vector.tensor_tensor(out_ap(0), t0[:], t1[:], mybir.AluOpType.subtract)
    t2 = tmp_pool.tile([32, W], F32, tag="rope_t0")
    t3 = tmp_pool.tile([32, W], F32, tag="rope_t1")
    nc.vector.tensor_mul(t2[:], src_ap(1), cos_hi)
    nc.vector.tensor_mul(t3[:], src_ap(0), sin_hi)
    nc.vector.tensor_tensor(out_ap(1), t2[:], t3[:], mybir.AluOpType.add)


def build_nc():
    nc = bacc.Bacc(None, target_bir_lowering=False, debug=False)
    dt_in = {}

    def din(name, shape, dt=BF16):
        t = nc.dram_tensor(name, list(shape), dt, kind="ExternalInput")
        dt_in[name] = t
        return t

    xT = din("xT", (DIM, TL))
    cosT_c = din("cosT_c", (HDR, TL))
    sinT_c = din("sinT_c", (HDR, TL))
    cosT_f = din("cosT_f", (HDR, S))
    sinT_f = din("sinT_f", (HDR, S))
    WdqT = din("WdqT", (DIM, DCQ))
    WdkvT = din("WdkvT", (DIM, DCKV))
    WkrT = din("WkrT", (DIM, HDR))
    WqmT = din("WqmT", (DCQ, HPC * 128))    # q main, per-head 128 cols
    WqeT = din("WqeT", (DCQ, HPC * 64))     # q ext
    WqrT = din("WqrT", (DCQ, HPC * 64))     # q rope (pre-summed, scaled)
    WkmT = din("WkmT", (DCKV, HPC * 128))
    WkeT = din("WkeT", (DCKV, HPC * 64))
    WvT = din("WvT", (DCKV, HPC * 128))
    WoT = din("WoT", (DIM, DIM))
    outT = nc.dram_tensor("outT", [DIM, TL], F32, kind="ExternalOutput")
    dbg = {}
    if DEBUG:
        dbg["bund"] = nc.dram_tensor("dbg_bund", [R, BUND, TL], F32, kind="ExternalOutput")
        dbg["q1"] = nc.dram_tensor("dbg_q1", [128, R * TL], F32, kind="ExternalOutput")
        dbg["q2"] = nc.dram_tensor("dbg_q2", [128, R * TL], F32, kind="ExternalOutput")
        dbg["k1"] = nc.dram_tensor("dbg_k1", [128, R * TL], F32, kind="ExternalOutput")
        dbg["k2"] = nc.dram_tensor("dbg_k2", [128, R * TL], F32, kind="ExternalOutput")
        dbg["v"] = nc.dram_tensor("dbg_v", [128, 32, HPC * 128], F32, kind="ExternalOutput")
        dbg["y"] = nc.dram_tensor("dbg_y", [R, HPC * 128, TL], F32, kind="ExternalOutput")

    with tile.TileContext(nc) as tc:
        with tc.tile_pool(name="const", bufs=1) as const, \
             tc.tile_pool(name="dram", bufs=1, space="DRAM") as dram:
            # --- constants ---
            ones = const.tile([128, 1], BF16, tag="ones")
            nc.gpsimd.memset(ones[:], 1.0)
            masks = []
            for s in range(4):  # keep iff y >= p + s*128  (y: free, p: partition)
                m = const.tile([128, 512], BF16, tag=f"mask{s}")
                nc.gpsimd.memset(m[:], 1.0)
                nc.gpsimd.affine_select(out=m[:], in_=m[:],
                                        compare_op=mybir.AluOpType.is_ge, fill=0.0,
                                        base=-s * 128, pattern=[[1, 512]],
                                        channel_multiplier=-1)
                masks.append(m)
            cosf = const.tile([HDR, S], BF16, tag="cosf")
            sinf = const.tile([HDR, S], BF16, tag="sinf")
            nc.sync.dma_start(cosf[:], cosT_f[:])
            nc.sync.dma_start(sinf[:], sinT_f[:])

            # --- collective buffers ---
            bounce = dram.tile([BUND, TL], BF16, tag="bounce")
            gath = dram.tile([R, BUND, TL], BF16, tag="gath", addr_space="Shared")
            a2a_in = dram.tile([R, HPC * 128, TL], BF16, tag="a2a_in")
            a2a_out = dram.tile([R, HPC * 128, TL], BF16, tag="a2a_out")

            # ================= Phase A: local down-projections =================
            with tc.tile_pool(name="paw", bufs=3) as paw, \
                 tc.tile_pool(name="pas", bufs=3) as pas, \
                 tc.tile_pool(name="pax", bufs=1) as pax, \
                 tc.tile_pool(name="paps", bufs=2, space="PSUM") as paps:
                xsb = pax.tile([128, NKD, TL], BF16, tag="xsb")
                nc.sync.dma_start(xsb[:], xT.rearrange("(ko p) t -> p ko t", p=128))
                csb = pax.tile([HDR, TL], BF16, tag="cckr")
                ssb = pax.tile([HDR, TL], BF16, tag="sskr")
                nc.sync.dma_start(csb[:], cosT_c[:])
                nc.sync.dma_start(ssb[:], sinT_c[:])

                def proj_a(WT, m, rows_off):
                    """one 128-row chunk of a down-proj -> bounce[rows_off...]"""
                    ps = paps.tile([128, TL], F32, tag="ps")
                    for k in range(NKD):
                        wt = paw.tile([128, 128], BF16, tag="wA")
                        nc.sync.dma_start(wt[:], WT[k * 128:(k + 1) * 128,
                                                    m * 128:(m + 1) * 128])
                        nc.tensor.matmul(ps[:], wt[:], xsb[:, k, :],
                                         start=(k == 0), stop=(k == NKD - 1))
                    ev = pas.tile([128, TL], BF16, tag="evA")
                    nc.scalar.copy(ev[:], ps[:])
                    nc.sync.dma_start(bounce[rows_off:rows_off + 128, :], ev[:])

                for m in range(NKQ):
                    proj_a(WdqT, m, m * 128)
                for m in range(NKC):
                    proj_a(WdkvT, m, DCQ + m * 128)
                # k_r: 64 rows + rope
                ps = paps.tile([64, TL], F32, tag="pskr")
                for k in range(NKD):
                    wt = paw.tile([128, HDR], BF16, tag="wKR")
                    nc.sync.dma_start(wt[:], WkrT[k * 128:(k + 1) * 128, :])
                    nc.tensor.matmul(ps[:], wt[:], xsb[:, k, :],
                                     start=(k == 0), stop=(k == NKD - 1))
                krr = pas.tile([64, TL], BF16, tag="krr")
                _rope_rows(nc,
                           lambda i: krr[i * 32:(i + 1) * 32, :],
                           lambda i: ps[i * 32:(i + 1) * 32, :],
                           csb[0:32, :], csb[32:64, :], ssb[0:32, :], ssb[32:64, :],
                           pas)
                nc.sync.dma_start(bounce[DCQ + DCKV:BUND, :], krr[:])

            nc.gpsimd.collective_compute(
                "AllGather", mybir.AluOpType.bypass,
                replica_groups=[list(range(R))],
                ins=[bounce.opt()], outs=[gath.opt()])
            if DEBUG:
                gsb = const.tile([128, R * BUND // 128 * TL // 512, 512], BF16, tag="dbgg")
                # simpler: dma gath -> dbg via f32 staging per chunk
                for r in range(R):
                    for m in range(BUND // 64):
                        st = const.tile([64, TL], BF16, tag="dbgst")
                        nc.sync.dma_start(st[:], gath[r, m * 64:(m + 1) * 64, :])
                        stf = const.tile([64, TL], F32, tag="dbgstf")
                        nc.vector.tensor_copy(stf[:], st[:])
                        nc.sync.dma_start(dbg["bund"][r, m * 64:(m + 1) * 64, :], stf[:])

            # ================= Phase B: per-head q/k/v =================
            with tc.tile_pool(name="pbw", bufs=1) as pbw, \
                 tc.tile_pool(name="pbc", bufs=2) as pbc, \
                 tc.tile_pool(name="pbig", bufs=1) as pbig, \
                 tc.tile_pool(name="pbe", bufs=3) as pbe:
                # resident per-head weights
                wqm = pbw.tile([128, NKQ, HPC * 128], BF16, tag="wqm")
                nc.sync.dma_start(wqm[:], WqmT.rearrange("(ko p) n -> p ko n", p=128))
                wqe = pbw.tile([128, NKQ, HPC * 64], BF16, tag="wqe")
                nc.sync.dma_start(wqe[:], WqeT.rearrange("(ko p) n -> p ko n", p=128))
                wqr = pbw.tile([128, NKQ, HPC * 64], BF16, tag="wqr")
                nc.sync.dma_start(wqr[:], WqrT.rearrange("(ko p) n -> p ko n", p=128))
                wkm = pbw.tile([128, NKC, HPC * 128], BF16, tag="wkm")
                nc.sync.dma_start(wkm[:], WkmT.rearrange("(ko p) n -> p ko n", p=128))
                wke = pbw.tile([128, NKC, HPC * 64], BF16, tag="wke")
                nc.sync.dma_start(wke[:], WkeT.rearrange("(ko p) n -> p ko n", p=128))
                wv = pbw.tile([128, NKC, HPC * 128], BF16, tag="wv")
                nc.sync.dma_start(wv[:], WvT.rearrange("(ko p) n -> p ko n", p=128))

                # persistent activations (feature-major; free dims [rt, 512])
                Q1 = [pbig.tile([128, R, TL], BF16, tag=f"Q1_{h}", name=f"Q1_{h}")
                      for h in range(HPC)]
                Q2 = [pbig.tile([128, R, TL], BF16, tag=f"Q2_{h}", name=f"Q2_{h}")
                      for h in range(HPC)]
                K1 = [pbig.tile([128, R, TL], BF16, tag=f"K1_{h}", name=f"K1_{h}")
                      for h in range(HPC)]
                K2 = [pbig.tile([128, R, TL], BF16, tag=f"K2_{h}", name=f"K2_{h}")
                      for h in range(HPC)]
                V = pbig.tile([128, 32, HPC * 128], BF16, tag="V")  # [j%128, j//128, e]

                import contextlib
                _pstk = contextlib.ExitStack()
                pbps = _pstk.enter_context(tc.tile_pool(name="pbps", bufs=2, space="PSUM"))
                pbp2 = _pstk.enter_context(tc.tile_pool(name="pbp2", bufs=2, space="PSUM"))
                for rt in range(R):
                    pos = (rt % 4) * TL  # position offset within the batch
                    cq = pbc.tile([128, NKQ, TL], BF16, tag="cqcol")
                    nc.sync.dma_start(cq[:], gath[rt, 0:DCQ, :]
                                      .rearrange("(ko p) t -> p ko t", p=128))
                    ckv = pbc.tile([128, NKC, TL], BF16, tag="ckvcol")
                    nc.sync.dma_start(ckv[:], gath[rt, DCQ:DCQ + DCKV, :]
                                      .rearrange("(ko p) t -> p ko t", p=128))
                    # q main per head
                    for h in range(HPC):
                        ps = pbps.tile([128, TL], F32, tag="psb")
                        for k in range(NKQ):
                            nc.tensor.matmul(ps[:], wqm[:, k, h * 128:(h + 1) * 128],
                                             cq[:, k, :], start=(k == 0),
                                             stop=(k == NKQ - 1))
                        nc.scalar.copy(Q1[h][:, rt, :], ps[:])
                    # q ext (both heads in one matmul chain)
                    ps = pbps.tile([128, TL], F32, tag="psb")
                    for k in range(NKQ):
                        nc.tensor.matmul(ps[:], wqe[:, k, :], cq[:, k, :],
                                         start=(k == 0), stop=(k == NKQ - 1))
                    for h in range(HPC):
                        nc.scalar.copy(Q2[h][0:64, rt, :], ps[h * 64:(h + 1) * 64, :])
                    # q rope (both heads), rope applied from psum
                    ps = pbps.tile([128, TL], F32, tag="psb")
                    for k in range(NKQ):
                        nc.tensor.matmul(ps[:], wqr[:, k, :], cq[:, k, :],
                                         start=(k == 0), stop=(k == NKQ - 1))
                    for h in range(HPC):
                        off = h * 64
                        _rope_rows(nc,
                                   lambda i, h=h: Q2[h][64 + i * 32:64 + (i + 1) * 32, rt, :],
                                   lambda i, off=off: ps[off + i * 32:off + (i + 1) * 32, :],
                                   cosf[0:32, pos:pos + TL], cosf[32:64, pos:pos + TL],
                                   sinf[0:32, pos:pos + TL], sinf[32:64, pos:pos + TL],
                                   pbe)
                    # k main per head
                    for h in range(HPC):
                        ps = pbps.tile([128, TL], F32, tag="psb")
                        for k in range(NKC):
                            nc.tensor.matmul(ps[:], wkm[:, k, h * 128:(h + 1) * 128],
                                             ckv[:, k, :], start=(k == 0),
                                             stop=(k == NKC - 1))
                        nc.scalar.copy(K1[h][:, rt, :], ps[:])
                    # k ext (both heads)
                    ps = pbps.tile([128, TL], F32, tag="psb")
                    for k in range(NKC):
                        nc.tensor.matmul(ps[:], wke[:, k, :], ckv[:, k, :],
                                         start=(k == 0), stop=(k == NKC - 1))
                    for h in range(HPC):
                        nc.scalar.copy(K2[h][0:64, rt, :], ps[h * 64:(h + 1) * 64, :])
                    # k rope rows: gathered k_r copy (same for both heads)
                    krg = pbc.tile([64, TL], BF16, tag="krg")
                    nc.sync.dma_start(krg[:], gath[rt, DCQ + DCKV:BUND, :])
                    for h in range(HPC):
                        nc.vector.tensor_copy(K2[h][64:128, rt, :], krg[:])
                    # v token-major: [j, e] = ckv-col as lhsT
                    for js in range(4):
                        ps = pbp2.tile([128, HPC * 128], F32, tag="psv")
                        for k in range(NKC):
                            nc.tensor.matmul(ps[:], ckv[:, k, js * 128:(js + 1) * 128],
                                             wv[:, k, :], start=(k == 0),
                                             stop=(k == NKC - 1))
                        nc.scalar.copy(V[:, rt * 4 + js, :], ps[:])

                if DEBUG:
                    for nm, tl_ in (("q1", Q1[0]), ("q2", Q2[0]), ("k1", K1[0]), ("k2", K2[0])):
                        st = pbc.tile([128, R * TL], F32, tag="dbgf")
                        nc.vector.tensor_copy(st[:], tl_[:].rearrange("p r t -> p (r t)"))
                        nc.sync.dma_start(dbg[nm][:], st[:])
                    stv = pbc.tile([128, 32, HPC * 128], F32, tag="dbgv")
                    nc.vector.tensor_copy(stv[:], V[:])
                    nc.sync.dma_start(dbg["v"][:], stv[:])

                _pstk.close()
                # ================= attention =================
                with tc.tile_pool(name="pat", bufs=6) as pat, \
                     tc.tile_pool(name="pan", bufs=2) as pan, \
                     tc.tile_pool(name="psS", bufs=3, space="PSUM") as psS, \
                     tc.tile_pool(name="psY", bufs=2, space="PSUM") as psY, \
                     tc.tile_pool(name="psL", bufs=2, space="PSUM") as psL:
                    for b in range(B):
                        for h in range(HPC):
                            for it in range(4):
                                rti = b * 4 + it
                                nj = 4 * (it + 1)
                                py = psY.tile([128, TL], F32, tag="py")
                                pl = psL.tile([1, TL], F32, tag="pl")
                                for j in range(nj):
                                    rtj = b * 4 + j // 4
                                    sub = j % 4
                                    sl = slice(sub * 128, (sub + 1) * 128)
                                    pss = psS.tile([128, TL], F32, tag="pss")
                                    nc.tensor.matmul(pss[:], K1[h][:, rtj, sl],
                                                     Q1[h][:, rti, :],
                                                     start=True, stop=False)
                                    nc.tensor.matmul(pss[:], K2[h][:, rtj, sl],
                                                     Q2[h][:, rti, :],
                                                     start=False, stop=True)
                                    et = pat.tile([128, TL], BF16, tag="et")
                                    s = j - 4 * it
                                    if s >= 0:  # diagonal chunk: mask after exp
                                        er = pat.tile([128, TL], BF16, tag="er")
                                        nc.scalar.activation(
                                            er[:], pss[:], mybir.ActivationFunctionType.Exp)
                                        nc.vector.tensor_mul(et[:], er[:], masks[s][:])
                                    else:
                                        nc.scalar.activation(
                                            et[:], pss[:], mybir.ActivationFunctionType.Exp)
                                    jj = b * 16 + j
                                    nc.tensor.matmul(py[:], V[:, jj, h * 128:(h + 1) * 128],
                                                     et[:], start=(j == 0),
                                                     stop=(j == nj - 1))
                                    nc.tensor.matmul(pl[:], ones[:], et[:],
                                                     start=(j == 0), stop=(j == nj - 1))
                                rec = pan.tile([1, TL], F32, tag="rec")
                                nc.vector.reciprocal(rec[:], pl[:])
                                rb = pan.tile([128, TL], F32, tag="rb")
                                nc.gpsimd.partition_broadcast(rb[:], rec[:])
                                yt = pan.tile([128, TL], BF16, tag="yt")
                                nc.vector.tensor_mul(yt[:], py[:], rb[:])
                                nc.sync.dma_start(
                                    a2a_in[rti, h * 128:(h + 1) * 128, :], yt[:])
                                if DEBUG:
                                    ytf = pan.tile([128, TL], F32, tag="ytf")
                                    nc.vector.tensor_copy(ytf[:], yt[:])
                                    nc.sync.dma_start(
                                        dbg["y"][rti, h * 128:(h + 1) * 128, :], ytf[:])

            nc.gpsimd.collective_compute(
                "AllToAll", mybir.AluOpType.bypass,
                replica_groups=[list(range(R))],
                ins=[a2a_in.opt()], outs=[a2a_out.opt()])

            # ================= Phase C: out projection =================
            with tc.tile_pool(name="pcw", bufs=4) as pcw, \
                 tc.tile_pool(name="pcy", bufs=1) as pcy, \
                 tc.tile_pool(name="pce", bufs=2) as pce, \
                 tc.tile_pool(name="pcps", bufs=2, space="PSUM") as pcps:
                ysb = pcy.tile([128, NKD, TL], BF16, tag="ysb")
                for ke in range(NKD):
                    nc.sync.dma_start(
                        ysb[:, ke, :],
                        a2a_out[ke % 8, (ke // 8) * 128:(ke // 8) * 128 + 128, :])
                for mo in range(NKD):
                    ps = pcps.tile([128, TL], F32, tag="pso")
                    for ke in range(NKD):
                        wt = pcw.tile([128, 128], BF16, tag="wO")
                        nc.sync.dma_start(wt[:], WoT[ke * 128:(ke + 1) * 128,
                                                     mo * 128:(mo + 1) * 128])
                        nc.tensor.matmul(ps[:], wt[:], ysb[:, ke, :],
                                         start=(ke == 0), stop=(ke == NKD - 1))
                    ev = pce.tile([128, TL], F32, tag="evO")
                    nc.scalar.copy(ev[:], ps[:])
                    nc.sync.dma_start(outT[mo * 128:(mo + 1) * 128, :], ev[:])

    nc.compile()
    return nc


def _prep_inputs(inputs):
    """Host-side sharding: returns in_maps list of 8 dicts."""
    x = np.asarray(inputs["x"], np.float32)
    cos = np.asarray(inputs["freq_cos"], np.float32)
    sin = np.asarray(inputs["freq_sin"], np.float32)
    for bn in ("b_dq", "b_uq", "b_qr", "b_dkv", "b_uk", "b_uv", "b_kr", "b_out"):
        assert np.abs(np.asarray(inputs[bn])).max() == 0.0, f"{bn} nonzero"
    W_dq = np.asarray(inputs["W_dq"], np.float32)
    W_uq = np.asarray(inputs["W_uq"], np.float32)
    W_qr = np.asarray(inputs["W_qr"], np.float32)
    W_dkv = np.asarray(inputs["W_dkv"], np.float32)
    W_uk = np.asarray(inputs["W_uk"], np.float32)
    W_uv = np.asarray(inputs["W_uv"], np.float32)
    W_kr = np.asarray(inputs["W_kr"], np.float32)
    W_out = np.asarray(inputs["W_out"], np.float32)

    scale = 1.0 / np.float32(np.sqrt(HD + HDR))
    xf = x.reshape(T, DIM)
    cosT = np.ascontiguousarray(cos.T).astype(BF)   # [64, 2048]
    sinT = np.ascontiguousarray(sin.T).astype(BF)
    WdqT = np.ascontiguousarray(W_dq.T).astype(BF)
    WdkvT = np.ascontiguousarray(W_dkv.T).astype(BF)
    WkrT = np.ascontiguousarray(W_kr.T).astype(BF)
    WoT = np.ascontiguousarray(W_out.T).astype(BF)

    # per-head main/ext/rope weight rows (fp32, scaled for q)
    def head_parts(h):
        qm = np.zeros((128, DCQ), np.float32)
        qe = np.zeros((64, DCQ), np.float32)
        qr = np.zeros((64, DCQ), np.float32)
        km = np.zeros((128, DCKV), np.float32)
        ke = np.zeros((64, DCKV), np.float32)
        c0 = 192 * h
        if h <= 9:
            qm[:] = W_uq[c0:c0 + 128]
            qe[:] = W_uq[c0 + 128:c0 + 192]
            km[:] = W_uk[c0:c0 + 128]
            ke[:] = W_uk[c0 + 128:c0 + 192]
        elif h == 10:
            qm[:] = W_uq[1920:2048]
            km[:] = W_uk[1920:2048]
            qr[:] = W_qr[0:64]
        else:
            r0 = 192 * h - 2048  # rope col start, multiple of 64
            for blk in range(3):
                qr += W_qr[r0 + blk * 64: r0 + (blk + 1) * 64]
        return qm * scale, qe * scale, qr * scale, km, ke

    in_maps = []
    for c in range(R):
        heads = (c, c + 8)
        qms, qes, qrs, kms, kes = zip(*(head_parts(h) for h in heads))
        WqmT = np.ascontiguousarray(np.concatenate(qms, 0).T).astype(BF)
        WqeT = np.ascontiguousarray(np.concatenate(qes, 0).T).astype(BF)
        WqrT = np.ascontiguousarray(np.concatenate(qrs, 0).T).astype(BF)
        WkmT = np.ascontiguousarray(np.concatenate(kms, 0).T).astype(BF)
        WkeT = np.ascontiguousarray(np.concatenate(kes, 0).T).astype(BF)
        WvT = np.ascontiguousarray(
            np.concatenate([W_uv[128 * h:128 * (h + 1)] for h in heads], 0).T).astype(BF)
        p0 = (c % 4) * TL
        in_maps.append({
            "xT": np.ascontiguousarray(xf[c * TL:(c + 1) * TL].T).astype(BF),
            "cosT_c": np.ascontiguousarray(cosT[:, p0:p0 + TL]),
            "sinT_c": np.ascontiguousarray(sinT[:, p0:p0 + TL]),
            "cosT_f": cosT, "sinT_f": sinT,
            "WdqT": WdqT, "WdkvT": WdkvT, "WkrT": WkrT,
            "WqmT": WqmT, "WqeT": WqeT, "WqrT": WqrT,
            "WkmT": WkmT, "WkeT": WkeT, "WvT": WvT, "WoT": WoT,
        })
    return in_maps


_NC_CACHE = {}


def get_nc():
    if "nc" not in _NC_CACHE:
        _NC_CACHE["nc"] = build_nc()
    return _NC_CACHE["nc"]


def kernel(**inputs) -> np.ndarray:
    nc = get_nc()
    in_maps = _prep_inputs(inputs)
    res = run_bass_kernel_spmd(nc, in_maps, core_ids=list(range(R)))
    out = np.empty((T, DIM), np.float32)
    for c in range(R):
        out[c * TL:(c + 1) * TL] = res.results[c]["outT"].T
    if DEBUG:
        kernel.debug_results = res.results
    return out.reshape(B, S, DIM)


# revision 15
# speedup vs baseline: 4838.0317x; 4838.0317x over previous
"""MLA (multi-head latent attention) Trainium2 kernel, 8-core SPMD.

Strategy (hardcoded for B=2, S=2048, DIM=2048, NH=16, HD=128, HDR=64,
DCKV=512, DCQ=1536):
  - Token-shard (flattened b*s, 512 tok/core) the low-rank down-projections
    (dq/dkv/kr + rope on kr), all feature-major (transposed) so matmuls need
    no on-device transposes.
  - AllGather the bundle [c_qT | c_kvT | k_rT] (bf16).
  - Head-shard: core c owns attention heads {c, c+8}. The reference views
    concat([uq, r_q]) as 16 heads x 192 dims, so head h's q/k window is
    cols [192h, 192h+192) of the concat. We decompose each head uniformly:
    main 128 dims + ext 64 dims + rope 64 dims, with host-built (possibly
    zero) weight slices; rope-only heads get the 3 relevant W_qr 64-row
    blocks PRE-SUMMED (valid because r_k is broadcast across rope blocks
    and rope is linear + identical per block).
  - Transpose-free attention: scoresT [k-tokens x q-tokens], exp without
    max-subtraction (scores are O(1) for this problem's scale), causal mask
    by 0/1 bf16 multiply, row-sums via ones-matmul, normalize after AV.
  - AllToAll y back to token sharding, local out-proj with full W_out.
  - bf16 matmul inputs (fp32 matmul is 4x slower on TRN2), fp32 PSUM.
"""
import sys

sys.path.insert(0, "/opt/trn_rl_repo")

import numpy as np
import ml_dtypes

import concourse.bass as bass
import concourse.mybir as mybir
import concourse.tile as tile
from concourse import bacc
from concourse.bass_utils import run_bass_kernel_spmd

BF = ml_dtypes.bfloat16
F32 = mybir.dt.float32
BF16 = mybir.dt.bfloat16

B, S, DIM = 2, 2048, 2048
NH, HD, HDR = 16, 128, 64
DCKV, DCQ = 512, 1536
R = 8            # cores
TL = 512         # tokens per core (flattened B*S / R)
T = B * S        # 4096
HPC = 2          # heads per core: {c, c+8}
NKQ = DCQ // 128   # 12 contraction chunks for c_q
NKD = DIM // 128   # 16 for x
NKC = DCKV // 128  # 4 for c_kv
BUND = DCQ + DCKV + HDR  # 2112 bundle rows

DEBUG = False


def _rope_rows(nc, out_ap, src_ap, cos_lo, cos_hi, sin_lo, sin_hi, tmp_pool):
    """rope on 64 feature-major rows: src/out [64, W] (out may be 2 slices).
    out[0:32]  = src[0:32]*cos_lo - src[32:64]*sin_lo
    out[32:64] = src[32:64]*cos_hi + src[0:32]*sin_hi
    src_ap: callable idx->AP for row slices (so PSUM sources work);
    out_ap: callable likewise."""
    W = cos_lo.shape[-1]
    t0 = tmp_pool.tile([32, W], F32, tag="rope_t0")
    t1 = tmp_pool.tile([32, W], F32, tag="rope_t1")
    nc.vector.tensor_mul(t0[:], src_ap(0), cos_lo)
    nc.vector.tensor_mul(t1[:], src_ap(1), sin_lo)
    nc.vector.tensor_tensor(out_ap(0), t0[:], t1[:], mybir.AluOpType.subtract)
    t2 = tmp_pool.tile([32, W], F32, tag="rope_t0")
    t3 = tmp_pool.tile([32, W], F32, tag="rope_t1")
    nc.vector.tensor_mul(t2[:], src_ap(1), cos_hi)
    nc.vector.tensor_mul(t3[:], src_ap(0), sin_hi)
    nc.vector.tensor_tensor(out_ap(1), t2[:], t3[:], mybir.AluOpType.add)


def build_nc(reps=1):
    nc = bacc.Bacc(None, target_bir_lowering=False, debug=False)
    dt_in = {}

    def din(name, shape, dt=BF16):
        t = nc.dram_tensor(name, list(shape), dt, kind="ExternalInput")
        dt_in[name] = t
        return t

    xT = din("xT", (DIM, TL))
    cosT_c = din("cosT_c", (HDR, TL))
    sinT_c = din("sinT_c", (HDR, TL))
    cosT_f = din("cosT_f", (HDR, S))
    sinT_f = din("sinT_f", (HDR, S))
    WdqT = din("WdqT", (DIM, DCQ))
    WdkvT = din("WdkvT", (DIM, DCKV))
    WkrT = din("WkrT", (DIM, HDR))
    WqmT = din("WqmT", (DCQ, HPC * 128))    # q main, per-head 128 cols
    WqeT = din("WqeT", (DCQ, HPC * 64))     # q ext
    WqrT = din("WqrT", (DCQ, HPC * 64))     # q rope (pre-summed, scaled)
    WkmT = din("WkmT", (DCKV, HPC * 128))
    WkeT = din("WkeT", (DCKV, HPC * 64))
    WvT = din("WvT", (DCKV, HPC * 128))
    WoT = din("WoT", (DIM, DIM))
    outT = nc.dram_tensor("outT", [DIM, TL], F32, kind="ExternalOutput")
    dbg = {}
    if DEBUG:
        dbg["bund"] = nc.dram_tensor("dbg_bund", [R, BUND, TL], F32, kind="ExternalOutput")
        dbg["q1"] = nc.dram_tensor("dbg_q1", [128, R * TL], F32, kind="ExternalOutput")
        dbg["q2"] = nc.dram_tensor("dbg_q2", [128, R * TL], F32, kind="ExternalOutput")
        dbg["k1"] = nc.dram_tensor("dbg_k1", [128, R * TL], F32, kind="ExternalOutput")
        dbg["k2"] = nc.dram_tensor("dbg_k2", [128, R * TL], F32, kind="ExternalOutput")
        dbg["v"] = nc.dram_tensor("dbg_v", [128, 32, HPC * 128], F32, kind="ExternalOutput")
        dbg["y"] = nc.dram_tensor("dbg_y", [R, HPC * 128, TL], F32, kind="ExternalOutput")

    with tile.TileContext(nc) as tc:
        with tc.tile_pool(name="const", bufs=1) as const, \
             tc.tile_pool(name="dram", bufs=1, space="DRAM") as dram:
            # --- constants ---
            ones = const.tile([128, 1], BF16, tag="ones")
            nc.gpsimd.memset(ones[:], 1.0)
            masks = []
            for s in range(4):  # keep iff y >= p + s*128  (y: free, p: partition)
                m = const.tile([128, 512], BF16, tag=f"mask{s}")
                nc.gpsimd.memset(m[:], 1.0)
                nc.gpsimd.affine_select(out=m[:], in_=m[:],
                                        compare_op=mybir.AluOpType.is_ge, fill=0.0,
                                        base=-s * 128, pattern=[[1, 512]],
                                        channel_multiplier=-1)
                masks.append(m)
            cosf = const.tile([HDR, S], BF16, tag="cosf")
            sinf = const.tile([HDR, S], BF16, tag="sinf")
            nc.sync.dma_start(cosf[:], cosT_f[:])
            nc.sync.dma_start(sinf[:], sinT_f[:])

            for _rep in range(reps):
                bounce = dram.tile([BUND, TL], BF16, tag=f"bounce{_rep}", name=f"bounce{_rep}")
                gath = dram.tile([R, BUND, TL], BF16, tag=f"gath{_rep}", name=f"gath{_rep}",
                                 addr_space="Shared")
                bounce_kv = bounce[0:DCKV + HDR, :]
                bounce_q = bounce[DCKV + HDR:BUND, :]
                gath_kv = gath[:, 0:DCKV + HDR, :]
                gath_q = gath[:, DCKV + HDR:BUND, :]
                a2a_in = dram.tile([R, HPC * 128, TL], BF16, tag=f"a2a_in{_rep}", name=f"a2a_in{_rep}")
                a2a_out = dram.tile([R, HPC * 128, TL], BF16, tag=f"a2a_out{_rep}", name=f"a2a_out{_rep}")
                _phase(nc, tc, const, ones, masks, cosf, sinf,
                       bounce, gath, bounce_kv, gath_kv, bounce_q, gath_q,
                       a2a_in, a2a_out, dt_in, outT, dbg, _rep)

    nc.compile()
    return nc


def _phase(nc, tc, const, ones, masks, cosf, sinf,
           bounce, gath, bounce_kv, gath_kv, bounce_q, gath_q,
           a2a_in, a2a_out, dt_in, outT, dbg, _rep):
    xT = dt_in["xT"]; cosT_c = dt_in["cosT_c"]; sinT_c = dt_in["sinT_c"]
    WdqT = dt_in["WdqT"]; WdkvT = dt_in["WdkvT"]; WkrT = dt_in["WkrT"]
    WqmT = dt_in["WqmT"]; WqeT = dt_in["WqeT"]; WqrT = dt_in["WqrT"]
    WkmT = dt_in["WkmT"]; WkeT = dt_in["WkeT"]; WvT = dt_in["WvT"]; WoT = dt_in["WoT"]

    # ================= Phase A: local down-projections =================
    with tc.tile_pool(name=f"paw{_rep}", bufs=1) as paw, \
         tc.tile_pool(name=f"pas{_rep}", bufs=3) as pas, \
         tc.tile_pool(name=f"pax{_rep}", bufs=1) as pax, \
         tc.tile_pool(name=f"paps{_rep}", bufs=2, space="PSUM") as paps:
        xsb = pax.tile([128, NKD, TL], BF16, tag="xsb")
        nc.sync.dma_start(xsb[:], xT.rearrange("(ko p) t -> p ko t", p=128))
        csb = pax.tile([HDR, TL], BF16, tag="cckr")
        ssb = pax.tile([HDR, TL], BF16, tag="sskr")
        nc.sync.dma_start(csb[:], cosT_c[:])
        nc.sync.dma_start(ssb[:], sinT_c[:])
        wdq = paw.tile([128, NKD, DCQ], BF16, tag="wdq")
        nc.sync.dma_start(wdq[:], WdqT.rearrange("(ko p) n -> p ko n", p=128))
        wdkv = paw.tile([128, NKD, DCKV], BF16, tag="wdkv")
        nc.sync.dma_start(wdkv[:], WdkvT.rearrange("(ko p) n -> p ko n", p=128))
        wkr = paw.tile([128, NKD, HDR], BF16, tag="wkr")
        nc.sync.dma_start(wkr[:], WkrT.rearrange("(ko p) n -> p ko n", p=128))

        def proj_a(wsb, m, rows_off, dst):
            ps = paps.tile([128, TL], F32, tag="ps", name="ps")
            for k in range(NKD):
                nc.tensor.matmul(ps[:], wsb[:, k, m * 128:(m + 1) * 128],
                                 xsb[:, k, :], start=(k == 0), stop=(k == NKD - 1))
            ev = pas.tile([128, TL], BF16, tag="evA", name="evA")
            nc.scalar.copy(ev[:], ps[:])
            nc.sync.dma_start(dst[rows_off:rows_off + 128, :], ev[:])

        for m in range(NKC):
            proj_a(wdkv, m, m * 128, bounce_kv)
        ps = paps.tile([64, TL], F32, tag="pskr")
        for k in range(NKD):
            nc.tensor.matmul(ps[:], wkr[:, k, :], xsb[:, k, :],
                             start=(k == 0), stop=(k == NKD - 1))
        krr = pas.tile([64, TL], BF16, tag="krr")
        _rope_rows(nc,
                   lambda i: krr[i * 32:(i + 1) * 32, :],
                   lambda i: ps[i * 32:(i + 1) * 32, :],
                   csb[0:32, :], csb[32:64, :], ssb[0:32, :], ssb[32:64, :], pas)
        nc.sync.dma_start(bounce_kv[DCKV:DCKV + HDR, :], krr[:])
        for m in range(NKQ):
            proj_a(wdq, m, m * 128, bounce_q)

    nc.gpsimd.collective_compute(
        "AllGather", mybir.AluOpType.bypass,
        replica_groups=[list(range(R))],
        ins=[bounce.opt()], outs=[gath.opt()])

    # ============ Phase B: per-head q/k/v + attention, per batch ============
    with tc.tile_pool(name=f"pbw{_rep}", bufs=1) as pbw, \
         tc.tile_pool(name=f"pbc{_rep}", bufs=3) as pbc, \
         tc.tile_pool(name=f"pbig{_rep}", bufs=1) as pbig, \
         tc.tile_pool(name=f"pbe{_rep}", bufs=3) as pbe, \
         tc.tile_pool(name=f"pbps{_rep}", bufs=2, space="PSUM") as pbps, \
         tc.tile_pool(name=f"pbp2{_rep}", bufs=1, space="PSUM") as pbp2, \
         tc.tile_pool(name=f"pat{_rep}", bufs=6) as pat, \
         tc.tile_pool(name=f"pan{_rep}", bufs=2) as pan, \
         tc.tile_pool(name=f"psS{_rep}", bufs=3, space="PSUM") as psS, \
         tc.tile_pool(name=f"psY{_rep}", bufs=1, space="PSUM") as psY, \
         tc.tile_pool(name=f"psL{_rep}", bufs=1, space="PSUM") as psL:
        wqm = pbw.tile([128, NKQ, HPC * 128], BF16, tag="wqm")
        nc.sync.dma_start(wqm[:], WqmT.rearrange("(ko p) n -> p ko n", p=128))
        wqe = pbw.tile([128, NKQ, HPC * 64], BF16, tag="wqe")
        nc.sync.dma_start(wqe[:], WqeT.rearrange("(ko p) n -> p ko n", p=128))
        wqr = pbw.tile([128, NKQ, HPC * 64], BF16, tag="wqr")
        nc.sync.dma_start(wqr[:], WqrT.rearrange("(ko p) n -> p ko n", p=128))
        wkm = pbw.tile([128, NKC, HPC * 128], BF16, tag="wkm")
        nc.sync.dma_start(wkm[:], WkmT.rearrange("(ko p) n -> p ko n", p=128))
        wke = pbw.tile([128, NKC, HPC * 64], BF16, tag="wke")
        nc.sync.dma_start(wke[:], WkeT.rearrange("(ko p) n -> p ko n", p=128))
        wv = pbw.tile([128, NKC, HPC * 128], BF16, tag="wv")
        nc.sync.dma_start(wv[:], WvT.rearrange("(ko p) n -> p ko n", p=128))

        Q1 = [pbig.tile([128, R, TL], BF16, tag=f"Q1_{h}", name=f"Q1_{h}")
              for h in range(HPC)]
        Q2 = [pbig.tile([128, R, TL], BF16, tag=f"Q2_{h}", name=f"Q2_{h}")
              for h in range(HPC)]
        K1 = [pbig.tile([128, R, TL], BF16, tag=f"K1_{h}", name=f"K1_{h}")
              for h in range(HPC)]
        K2 = [pbig.tile([128, R, TL], BF16, tag=f"K2_{h}", name=f"K2_{h}")
              for h in range(HPC)]
        V = pbig.tile([128, 32, HPC * 128], BF16, tag="V")

        def do_rt(rt):
            pos = (rt % 4) * TL
            ckv = pbc.tile([128, NKC, TL], BF16, tag="ckvcol", name="ckvcol")
            nc.sync.dma_start(ckv[:], gath[rt, 0:DCKV, :]
                              .rearrange("(ko p) t -> p ko t", p=128))
            # ---- k/v projections (depend on AG1 only) ----
            for h in range(HPC):
                ps = pbps.tile([128, TL], F32, tag="psb", name="psb")
                for k in range(NKC):
                    nc.tensor.matmul(ps[:], wkm[:, k, h * 128:(h + 1) * 128],
                                     ckv[:, k, :], start=(k == 0), stop=(k == NKC - 1))
                nc.scalar.copy(K1[h][:, rt, :], ps[:])
            ps = pbps.tile([128, TL], F32, tag="psb", name="psb")
            for k in range(NKC):
                nc.tensor.matmul(ps[:], wke[:, k, :], ckv[:, k, :],
                                 start=(k == 0), stop=(k == NKC - 1))
            for h in range(HPC):
                nc.scalar.copy(K2[h][0:64, rt, :], ps[h * 64:(h + 1) * 64, :])
            krg = pbc.tile([64, TL], BF16, tag="krg", name="krg")
            nc.sync.dma_start(krg[:], gath[rt, DCKV:DCKV + HDR, :])
            for h in range(HPC):
                nc.vector.tensor_copy(K2[h][64:128, rt, :], krg[:])
            for js in range(4):
                ps = pbp2.tile([128, HPC * 128], F32, tag="psv", name="psv")
                for k in range(NKC):
                    nc.tensor.matmul(ps[:], ckv[:, k, js * 128:(js + 1) * 128],
                                     wv[:, k, :], start=(k == 0), stop=(k == NKC - 1))
                nc.scalar.copy(V[:, rt * 4 + js, :], ps[:])
            # ---- q projections (depend on AG2) ----
            cq = pbc.tile([128, NKQ, TL], BF16, tag="cqcol", name="cqcol")
            nc.sync.dma_start(cq[:], gath[rt, DCKV + HDR:BUND, :]
                              .rearrange("(ko p) t -> p ko t", p=128))
            for h in range(HPC):
                ps = pbps.tile([128, TL], F32, tag="psb", name="psb")
                for k in range(NKQ):
                    nc.tensor.matmul(ps[:], wqm[:, k, h * 128:(h + 1) * 128],
                                     cq[:, k, :], start=(k == 0), stop=(k == NKQ - 1))
                nc.scalar.copy(Q1[h][:, rt, :], ps[:])
            ps = pbps.tile([128, TL], F32, tag="psb", name="psb")
            for k in range(NKQ):
                nc.tensor.matmul(ps[:], wqe[:, k, :], cq[:, k, :],
                                 start=(k == 0), stop=(k == NKQ - 1))
            for h in range(HPC):
                nc.scalar.copy(Q2[h][0:64, rt, :], ps[h * 64:(h + 1) * 64, :])
            ps = pbps.tile([128, TL], F32, tag="psb", name="psb")
            for k in range(NKQ):
                nc.tensor.matmul(ps[:], wqr[:, k, :], cq[:, k, :],
                                 start=(k == 0), stop=(k == NKQ - 1))
            for h in range(HPC):
                off = h * 64
                _rope_rows(nc,
                           lambda i, h=h: Q2[h][64 + i * 32:64 + (i + 1) * 32, rt, :],
                           lambda i, off=off: ps[off + i * 32:off + (i + 1) * 32, :],
                           cosf[0:32, pos:pos + TL], cosf[32:64, pos:pos + TL],
                           sinf[0:32, pos:pos + TL], sinf[32:64, pos:pos + TL], pbe)

        def do_attn(b, h):
            for it in range(4):
                rti = b * 4 + it
                nj = 4 * (it + 1)
                py = psY.tile([128, TL], F32, tag="py", name="py")
                pl = psL.tile([1, TL], F32, tag="pl", name="pl")
                for j in range(nj):
                    rtj = b * 4 + j // 4
                    sub = j % 4
                    sl = slice(sub * 128, (sub + 1) * 128)
                    pss = psS.tile([128, TL], F32, tag="pss", name="pss")
                    nc.tensor.matmul(pss[:], K1[h][:, rtj, sl], Q1[h][:, rti, :],
                                     start=True, stop=False)
                    nc.tensor.matmul(pss[:], K2[h][:, rtj, sl], Q2[h][:, rti, :],
                                     start=False, stop=True)
                    et = pat.tile([128, TL], BF16, tag="et", name="et")
                    s = j - 4 * it
                    if s >= 0:
                        er = pat.tile([128, TL], BF16, tag="er", name="er")
                        nc.scalar.activation(er[:], pss[:],
                                             mybir.ActivationFunctionType.Exp)
                        nc.vector.tensor_mul(et[:], er[:], masks[s][:])
                    else:
                        nc.scalar.activation(et[:], pss[:],
                                             mybir.ActivationFunctionType.Exp)
                    jj = b * 16 + j
                    nc.tensor.matmul(py[:], V[:, jj, h * 128:(h + 1) * 128], et[:],
                                     start=(j == 0), stop=(j == nj - 1))
                    nc.tensor.matmul(pl[:], ones[:], et[:],
                                     start=(j == 0), stop=(j == nj - 1))
                rec = pan.tile([1, TL], F32, tag="rec", name="rec")
                nc.vector.reciprocal(rec[:], pl[:])
                rb = pan.tile([128, TL], F32, tag="rb", name="rb")
                nc.gpsimd.partition_broadcast(rb[:], rec[:])
                yt = pan.tile([128, TL], BF16, tag="yt", name="yt")
                nc.vector.tensor_mul(yt[:], py[:], rb[:])
                nc.sync.dma_start(a2a_in[rti, h * 128:(h + 1) * 128, :], yt[:])

        for b in range(B):
            for rt in range(b * 4, b * 4 + 4):
                do_rt(rt)
            for h in range(HPC):
                do_attn(b, h)

    nc.gpsimd.collective_compute(
        "AllToAll", mybir.AluOpType.bypass,
        replica_groups=[list(range(R))],
        ins=[a2a_in.opt()], outs=[a2a_out.opt()])

    # ================= Phase C: out projection =================
    with tc.tile_pool(name=f"pcw{_rep}", bufs=1) as pcw, \
         tc.tile_pool(name=f"pcy{_rep}", bufs=1) as pcy, \
         tc.tile_pool(name=f"pce{_rep}", bufs=2) as pce, \
         tc.tile_pool(name=f"pcps{_rep}", bufs=2, space="PSUM") as pcps:
        ysb = pcy.tile([128, NKD, TL], BF16, tag="ysb")
        for ke in range(NKD):
            nc.sync.dma_start(
                ysb[:, ke, :],
                a2a_out[ke % 8, (ke // 8) * 128:(ke // 8) * 128 + 128, :])
        wo = pcw.tile([128, NKD, DIM], BF16, tag="wo")
        nc.sync.dma_start(wo[:], WoT.rearrange("(ko p) n -> p ko n", p=128))
        for mo in range(NKD):
            ps = pcps.tile([128, TL], F32, tag="pso", name="pso")
            for ke in range(NKD):
                nc.tensor.matmul(ps[:], wo[:, ke, mo * 128:(mo + 1) * 128],
                                 ysb[:, ke, :], start=(ke == 0), stop=(ke == NKD - 1))
            ev = pce.tile([128, TL], F32, tag="evO", name="evO")
            nc.scalar.copy(ev[:], ps[:])
            nc.sync.dma_start(outT[mo * 128:(mo + 1) * 128, :], ev[:])


def _prep_inputs(inputs):
    """Host-side sharding: returns in_maps list of 8 dicts."""
    x = np.asarray(inputs["x"], np.float32)
    cos = np.asarray(inputs["freq_cos"], np.float32)
    sin = np.asarray(inputs["freq_sin"], np.float32)
    for bn in ("b_dq", "b_uq", "b_qr", "b_dkv", "b_uk", "b_uv", "b_kr", "b_out"):
        assert np.abs(np.asarray(inputs[bn])).max() == 0.0, f"{bn} nonzero"
    W_dq = np.asarray(inputs["W_dq"], np.float32)
    W_uq = np.asarray(inputs["W_uq"], np.float32)
    W_qr = np.asarray(inputs["W_qr"], np.float32)
    W_dkv = np.asarray(inputs["W_dkv"], np.float32)
    W_uk = np.asarray(inputs["W_uk"], np.float32)
    W_uv = np.asarray(inputs["W_uv"], np.float32)
    W_kr = np.asarray(inputs["W_kr"], np.float32)
    W_out = np.asarray(inputs["W_out"], np.float32)

    scale = 1.0 / np.float32(np.sqrt(HD + HDR))
    xf = x.reshape(T, DIM)
    cosT = np.ascontiguousarray(cos.T).astype(BF)   # [64, 2048]
    sinT = np.ascontiguousarray(sin.T).astype(BF)
    WdqT = np.ascontiguousarray(W_dq.T).astype(BF)
    WdkvT = np.ascontiguousarray(W_dkv.T).astype(BF)
    WkrT = np.ascontiguousarray(W_kr.T).astype(BF)
    WoT = np.ascontiguousarray(W_out.T).astype(BF)

    # per-head main/ext/rope weight rows (fp32, scaled for q)
    def head_parts(h):
        qm = np.zeros((128, DCQ), np.float32)
        qe = np.zeros((64, DCQ), np.float32)
        qr = np.zeros((64, DCQ), np.float32)
        km = np.zeros((128, DCKV), np.float32)
        ke = np.zeros((64, DCKV), np.float32)
        c0 = 192 * h
        if h <= 9:
            qm[:] = W_uq[c0:c0 + 128]
            qe[:] = W_uq[c0 + 128:c0 + 192]
            km[:] = W_uk[c0:c0 + 128]
            ke[:] = W_uk[c0 + 128:c0 + 192]
        elif h == 10:
            qm[:] = W_uq[1920:2048]
            km[:] = W_uk[1920:2048]
            qr[:] = W_qr[0:64]
        else:
            r0 = 192 * h - 2048  # rope col start, multiple of 64
            for blk in range(3):
                qr += W_qr[r0 + blk * 64: r0 + (blk + 1) * 64]
        return qm * scale, qe * scale, qr * scale, km, ke

    in_maps = []
    for c in range(R):
        heads = (c, c + 8)
        qms, qes, qrs, kms, kes = zip(*(head_parts(h) for h in heads))
        WqmT = np.ascontiguousarray(np.concatenate(qms, 0).T).astype(BF)
        WqeT = np.ascontiguousarray(np.concatenate(qes, 0).T).astype(BF)
        WqrT = np.ascontiguousarray(np.concatenate(qrs, 0).T).astype(BF)
        WkmT = np.ascontiguousarray(np.concatenate(kms, 0).T).astype(BF)
        WkeT = np.ascontiguousarray(np.concatenate(kes, 0).T).astype(BF)
        WvT = np.ascontiguousarray(
            np.concatenate([W_uv[128 * h:128 * (h + 1)] for h in heads], 0).T).astype(BF)
        p0 = (c % 4) * TL
        in_maps.append({
            "xT": np.ascontiguousarray(xf[c * TL:(c + 1) * TL].T).astype(BF),
            "cosT_c": np.ascontiguousarray(cosT[:, p0:p0 + TL]),
            "sinT_c": np.ascontiguousarray(sinT[:, p0:p0 + TL]),
            "cosT_f": cosT, "sinT_f": sinT,
            "WdqT": WdqT, "WdkvT": WdkvT, "WkrT": WkrT,
            "WqmT": WqmT, "WqeT": WqeT, "WqrT": WqrT,
            "WkmT": WkmT, "WkeT": WkeT, "WvT": WvT, "WoT": WoT,
        })
    return in_maps


_NC_CACHE = {}


def get_nc(reps=1):
    if reps not in _NC_CACHE:
        _NC_CACHE[reps] = build_nc(reps)
    return _NC_CACHE[reps]


def kernel(**inputs) -> np.ndarray:
    nc = get_nc()
    in_maps = _prep_inputs(inputs)
    res = run_bass_kernel_spmd(nc, in_maps, core_ids=list(range(R)))
    out = np.empty((T, DIM), np.float32)
    for c in range(R):
        out[c * TL:(c + 1) * TL] = res.results[c]["outT"].T
    if DEBUG:
        kernel.debug_results = res.results
    return out.reshape(B, S, DIM)
